# revision 5
# baseline (speedup 1.0000x reference)
"""KMeans vq_codebook kernel for 8 trn2 NeuronCores.

Strategy (data-parallel over N, per sharding hint):
  - Host: split x and 2*centers into fp16 hi/lo pairs (x = xh + xl exactly
    in fp32; same for c2 = 2*centers). Transpose so the D=128 contraction
    sits on SBUF partitions. One-hot encode labels y (fp16).
  - Device (per core, N_shard = 32768 rows, 256 tiles of 128 rows):
      PE:  G2 = xh@ch + xh@cl + xl@ch   (3 full-rate fp16 matmuls, fp32
           PSUM accumulate; dropped xl@cl term is ~1e-5 abs — below fp32
           matmul noise)
      DVE (batched over 4 tiles): h = G2 - csq; hmax = max_k h;
           onehot = (h == hmax) as fp16
      PE:  conf[10, 256] += onehot_y_tile.T @ onehot   (accumulated in
           PSUM across all tiles; exact integer counts in fp32)
  - Host: loss = sum(x*x) (fp64) - sum(hmax); conf summed over cores
    -> acc = sum_k max_c conf / N.   (min_k dist = x_sq - max_k h.)
"""

import sys

sys.path.insert(0, "/opt/trn_rl_repo")

import numpy as np

import concourse.bass as bass
import concourse.mybir as mybir
import concourse.tile as tile
from concourse.bass_utils import run_bass_kernel_spmd

N_FULL = 262144
D = 128
K = 256
NUM_CORES = 8
NS = N_FULL // NUM_CORES  # 32768 rows per core
NUM_GT_CLASSES = 10
NGC = NUM_GT_CLASSES

F32 = mybir.dt.float32
F16 = mybir.dt.float16

_CACHE = {}
LAST = None  # BassKernelResults of the most recent run (for benchmarking)
HW_EXEC_NS = None  # per-execution device time measured by _bench_exec


def _bench_exec(nc, in_maps, n_cores):
    """Estimate per-execution device time of the compiled NEFF.

    Replicates bass2jax.run_bass_via_pjrt's jit(shard_map(custom_call))
    plumbing, but keeps the jitted callable and times pipelined repeated
    executions, reporting the marginal time per execution (large batch
    minus small batch, so fixed dispatch/pipeline-fill cost cancels).
    """
    import time

    import jax
    from jax.experimental.shard_map import shard_map
    from jax.sharding import Mesh, NamedSharding, PartitionSpec

    from concourse import bass2jax as b2j

    b2j.install_neuronx_cc_hook()
    partition_name = nc.partition_id_tensor.name if nc.partition_id_tensor else None
    in_names, out_names, out_avals, zero_outs = [], [], [], []
    for alloc in nc.m.functions[0].allocations:
        if not isinstance(alloc, mybir.MemoryLocationSet):
            continue
        name = alloc.memorylocations[0].name
        if alloc.kind == "ExternalInput":
            if name != partition_name:
                in_names.append(name)
        elif alloc.kind == "ExternalOutput":
            out_names.append(name)
            shape = tuple(alloc.tensor_shape)
            dtype = mybir.dt.np(alloc.dtype)
            out_avals.append(jax.core.ShapedArray(shape, dtype))
            zero_outs.append(np.zeros(shape, dtype))
    n_params = len(in_names)
    n_outs = len(out_avals)
    in_names.extend(out_names)
    if partition_name is not None:
        in_names.append(partition_name)
    donate = tuple(range(n_params, n_params + n_outs))

    def _body(*args):
        operands = list(args)
        if partition_name is not None:
            operands.append(b2j.partition_id_tensor())
        outs = b2j._bass_exec_p.bind(
            *operands,
            out_avals=tuple(out_avals),
            in_names=tuple(in_names),
            out_names=tuple(out_names),
            lowering_input_output_aliases=(),
            sim_require_finite=True,
            sim_require_nnan=True,
            nc=nc,
        )
        return tuple(outs)

    devices = jax.devices()[:n_cores]
    mesh = Mesh(np.asarray(devices), ("core",))
    in_specs = (PartitionSpec("core"),) * (n_params + n_outs)
    out_specs = (PartitionSpec("core"),) * len(out_names)
    sharded = jax.jit(
        shard_map(
            _body, mesh=mesh, in_specs=in_specs, out_specs=out_specs, check_rep=False
        ),
        donate_argnums=donate,
        keep_unused=True,
    )
    sh = NamedSharding(mesh, PartitionSpec("core"))
    concat_in = [
        jax.device_put(
            np.concatenate([np.asarray(m[name]) for m in in_maps], axis=0), sh
        )
        for name in in_names[:n_params]
    ]
    concat_zero = [
        np.zeros((n_cores * z.shape[0], *z.shape[1:]), z.dtype) for z in zero_outs
    ]

    def run_batch(iters):
        zs = [
            [jax.device_put(z, sh) for z in concat_zero] for _ in range(iters)
        ]
        for z in zs:
            for a in z:
                a.block_until_ready()
        t0 = time.perf_counter()
        outs = None
        for i in range(iters):
            outs = sharded(*concat_in, *zs[i])
        for o in outs:
            o.block_until_ready()
        return time.perf_counter() - t0

    run_batch(2)  # warm-up (compile + pipeline)
    best = None
    for _ in range(3):
        t_small = run_batch(4)
        t_large = run_batch(20)
        marginal = (t_large - t_small) / 16.0
        best = marginal if best is None else min(best, marginal)
    return int(best * 1e9)


def build_nc(ns=NS, supertile=2048, batch=4, for_sim=False):
    """Build the single-core Bass program (same program runs SPMD on 8 cores)."""
    ntiles = ns // 128
    n_super = ns // supertile
    tiles_per_super = supertile // 128
    assert tiles_per_super % batch == 0

    import concourse.bacc as bacc

    nc = bacc.Bacc("TRN2", target_bir_lowering=False, debug=bool(for_sim))

    xh_d = nc.declare_dram_parameter("xh", [D, ns], F16, isOutput=False)
    xl_d = nc.declare_dram_parameter("xl", [D, ns], F16, isOutput=False)
    ch_d = nc.declare_dram_parameter("ch", [D, K], F16, isOutput=False)
    cl_d = nc.declare_dram_parameter("cl", [D, K], F16, isOutput=False)
    csqb_d = nc.declare_dram_parameter("csqb", [D, K], F32, isOutput=False)
    yoh_d = nc.declare_dram_parameter("yoh", [128, NGC * ntiles], F16, isOutput=False)
    hmax_out = nc.declare_dram_parameter("hmax", [128, ntiles], F32, isOutput=True)
    conf_out = nc.declare_dram_parameter("conf", [NGC, K], F32, isOutput=True)

    with tile.TileContext(nc) as tc:
        with (
            tc.tile_pool(name="const", bufs=1) as constp,
            tc.tile_pool(name="xs", bufs=3) as xsp,
            tc.tile_pool(name="hb", bufs=3) as hbp,
            tc.tile_pool(name="acc", bufs=1) as accp,
            tc.tile_pool(name="ps", bufs=2, space=bass.MemorySpace.PSUM) as psp,
            tc.tile_pool(name="psconf", bufs=1, space=bass.MemorySpace.PSUM) as pscp,
        ):
            ch_t = constp.tile([D, K], F16, tag="ch")
            cl_t = constp.tile([D, K], F16, tag="cl")
            csq_t = constp.tile([D, K], F32, tag="csq")
            yoh_t = constp.tile([128, NGC * ntiles], F16, tag="yoh")
            nc.sync.dma_start(ch_t[:], ch_d[:, :])
            nc.sync.dma_start(cl_t[:], cl_d[:, :])
            nc.sync.dma_start(csq_t[:], csqb_d[:, :])
            nc.sync.dma_start(yoh_t[:], yoh_d[:, :])

            hmax_acc = accp.tile([128, ntiles], F32, tag="hmax")
            conf_ps = pscp.tile([NGC, K], F32, tag="conf")

            first_conf = True
            for st in range(n_super):
                xh_s = xsp.tile([D, supertile], F16, tag="xh")
                xl_s = xsp.tile([D, supertile], F16, tag="xl")
                nc.sync.dma_start(xh_s[:], xh_d[:, st * supertile : (st + 1) * supertile])
                nc.sync.dma_start(xl_s[:], xl_d[:, st * supertile : (st + 1) * supertile])
                for bb in range(tiles_per_super // batch):
                    jb = st * tiles_per_super + bb * batch  # global tile id of batch
                    g2 = psp.tile([128, batch, K], F32, tag="g2")
                    for q in range(batch):
                        sl = slice((bb * batch + q) * 128, (bb * batch + q + 1) * 128)
                        nc.tensor.matmul(
                            g2[:, q, :], xh_s[:, sl], ch_t[:], start=True, stop=False
                        )
                        nc.tensor.matmul(
                            g2[:, q, :], xh_s[:, sl], cl_t[:], start=False, stop=False
                        )
                        nc.tensor.matmul(
                            g2[:, q, :], xl_s[:, sl], ch_t[:], start=False, stop=True
                        )
                    # h = G2 - csq  (batched over `batch` tiles)
                    h4 = hbp.tile([128, batch, K], F32, tag="h4")
                    csq_b = csq_t[:].unsqueeze(1).broadcast_to([128, batch, K])
                    nc.vector.tensor_tensor(
                        h4[:], g2[:], csq_b, mybir.AluOpType.subtract
                    )
                    # hmax[:, jb:jb+batch] = max_k h
                    nc.vector.tensor_reduce(
                        hmax_acc[:, jb : jb + batch],
                        h4[:],
                        axis=mybir.AxisListType.X,
                        op=mybir.AluOpType.max,
                    )
                    # onehot = (h == hmax) in fp16
                    oh4 = hbp.tile([128, batch, K], F16, tag="oh4")
                    hmax_b = (
                        hmax_acc[:, jb : jb + batch]
                        .unsqueeze(2)
                        .broadcast_to([128, batch, K])
                    )
                    nc.vector.tensor_tensor(
                        oh4[:], h4[:], hmax_b, mybir.AluOpType.is_equal
                    )
                    # conf += yoh_j.T @ onehot_j
                    for q in range(batch):
                        j = jb + q
                        nc.tensor.matmul(
                            conf_ps[:],
                            yoh_t[:, NGC * j : NGC * (j + 1)],
                            oh4[:, q, :],
                            start=first_conf,
                            stop=(j == ntiles - 1),
                            skip_group_check=True,
                        )
                        first_conf = False

            conf_sb = accp.tile([NGC, K], F32, tag="confsb")
            nc.vector.tensor_copy(conf_sb[:], conf_ps[:])
            nc.sync.dma_start(hmax_out[:, :], hmax_acc[:])
            nc.sync.dma_start(conf_out[:, :], conf_sb[:])

    nc.compile()
    return nc


def _split16(a):
    hi = a.astype(np.float16)
    lo = (a - hi.astype(np.float32)).astype(np.float16)
    return hi, lo


def kernel(x, y, centers):
    x = np.asarray(x, dtype=np.float32)
    y_np = np.asarray(y).astype(np.int64)
    centers = np.asarray(centers, dtype=np.float32)
    n = x.shape[0]
    assert n == N_FULL and x.shape[1] == D and centers.shape == (K, D)

    if "nc" not in _CACHE:
        _CACHE["nc"] = build_nc()
    nc = _CACHE["nc"]

    ntiles = NS // 128

    xt = np.ascontiguousarray(x.T)  # [128, N] f32
    xh, xl = _split16(xt)
    c2t = np.ascontiguousarray(centers.T) * np.float32(2.0)  # [128, K]
    ch, cl = _split16(c2t)
    csq = np.sum(centers.astype(np.float64) ** 2, axis=1).astype(np.float32)
    csqb = np.ascontiguousarray(np.broadcast_to(csq[None, :], (D, K)))

    # One-hot labels, laid out per tile: yoh[p, 10*j + c] = (y[j*128+p] == c)
    y_cores = y_np.reshape(NUM_CORES, ntiles, 128)  # [core, tile j, p]
    oh = (y_cores[:, :, :, None] == np.arange(NGC)[None, None, None, :]).astype(
        np.float16
    )  # [core, j, p, c]
    yoh_all = np.ascontiguousarray(
        oh.transpose(0, 2, 1, 3).reshape(NUM_CORES, 128, ntiles * NGC)
    )

    in_maps = []
    for c in range(NUM_CORES):
        sl = slice(c * NS, (c + 1) * NS)
        in_maps.append(
            {
                "xh": np.ascontiguousarray(xh[:, sl]),
                "xl": np.ascontiguousarray(xl[:, sl]),
                "ch": ch,
                "cl": cl,
                "csqb": csqb,
                "yoh": yoh_all[c],
            }
        )

    kr = run_bass_kernel_spmd(nc, in_maps, list(range(NUM_CORES)))
    global LAST, HW_EXEC_NS
    LAST = kr
    res = kr.results

    import os

    if os.environ.get("BASS_BENCH") == "1":
        HW_EXEC_NS = _bench_exec(nc, in_maps, NUM_CORES)

    hmax_sum = 0.0
    conf = np.zeros((K, NGC), dtype=np.float64)
    for c in range(NUM_CORES):
        hmax_sum += float(np.asarray(res[c]["hmax"]).astype(np.float64).sum())
        conf += np.asarray(res[c]["conf"]).astype(np.float64).T  # [K, 10]

    x64 = x.astype(np.float64)
    x_sq_total = float(np.einsum("nd,nd->", x64, x64, optimize=True))
    loss = np.float32(x_sq_total - hmax_sum)

    correct_ct = conf.max(axis=1).sum()
    acc = np.float32(correct_ct / np.float32(n))
    return loss, acc

